# revision 5
# baseline (speedup 1.0000x reference)
"""MoE (top-2 of 8 experts + shared expert) Trainium2 Bass kernel, 8-core expert-parallel.

Sharding: expert-parallel — core e owns expert e's FFN (w1[e], w2[e]) and an
H-slice (256 rows) of the shared expert. Gating is replicated (cheap). Each
core produces a partial [N, C] output; the host sums the 8 partials.

Math per core e (dense over its own expert; g zero-masks non-routed tokens):
  xt = rmsnorm(x);  raw = xt @ [gate_w; shared_gate_w].T  (exact fp32 path)
  top-2 softmax gating -> ge[n] (this expert's combine weight, 0 if not routed)
  moe    = relu(xt @ w1[e].T)^2 @ w2[e].T * ge[n]
  shared = relu(xt @ k_w[sl].T + k_b[sl])^2 @ v_w[:,sl].T * sigmoid(raw[:,8])
  partial = moe + shared        (v_b handled on host; zero in practice)
aux_loss computed identically on every core; host takes core 0's.

Big matmuls run as float32r (4x fp32 PE rate, ~1.5e-4 relmax rounding); the
gating matmul runs in exact fp32 so top-2 selection cannot flip.

Memory plan (per partition): w1t resident (64KB); w2t streamed in [128,512]
slices, jj-outer accumulation, two token-chunks ("nm pair") per stream pass;
x/xt/h2/xtT transient. PSUM: ph/xpose pool (2 banks), po/psh pool (4), raw9 (1).
"""
import sys

for _p in ("/root/.axon_site", "/root/.axon_site/_ro/trn_rl_repo", "/opt/trn_rl_repo"):
    if _p not in sys.path:
        sys.path.append(_p)

import numpy as np

import concourse.bacc as bacc
import concourse.tile as tile
import concourse.mybir as mybir
from concourse.bass_utils import run_bass_kernel_spmd

F32 = mybir.dt.float32
F32R = mybir.dt.float32r
ALU = mybir.AluOpType
ACTF = mybir.ActivationFunctionType
AXX = mybir.AxisListType.X

N, C, H, E = 2048, 1024, 2048, 8
NCORES = 8
HS = H // NCORES          # shared-expert H slice per core (256)
TPC = 2                   # 128-token tiles per chunk (chunk = 256 tokens)
NM = N // (TPC * 128)     # 8 token chunks
NPAIR = NM // 2           # chunk pairs sharing one w2t stream pass
CC = C // 128             # 8 contraction chunks
JJ = H // 128             # 16 expert-H chunks
JS = HS // 128            # 2 shared-H chunks
EPS = 1.1920929e-07
AUX_SCALE = E * 0.01 / (N * N)


def build_program():
    nc = bacc.Bacc("TRN2", target_bir_lowering=False, debug=False, num_devices=NCORES)

    x_d = nc.dram_tensor("x", [N, C], F32, kind="ExternalInput").ap()
    w1t_d = nc.dram_tensor("w1t", [C, H], F32R, kind="ExternalInput").ap()
    w2t_d = nc.dram_tensor("w2t", [H, C], F32R, kind="ExternalInput").ap()
    kwt_d = nc.dram_tensor("kwt", [C, HS], F32R, kind="ExternalInput").ap()
    vwt_d = nc.dram_tensor("vwt", [HS, C], F32R, kind="ExternalInput").ap()
    kb_d = nc.dram_tensor("kb", [128, JS], F32, kind="ExternalInput").ap()
    gwt_d = nc.dram_tensor("gwt", [C, 9], F32, kind="ExternalInput").ap()
    sel_d = nc.dram_tensor("sel", [128, E], F32, kind="ExternalInput").ap()
    ident_d = nc.dram_tensor("ident", [128, 128], F32, kind="ExternalInput").ap()

    out_d = nc.dram_tensor("out", [N, C], F32, kind="ExternalOutput").ap()
    aux_d = nc.dram_tensor("aux", [1, 1], F32, kind="ExternalOutput").ap()
    sg_d = nc.dram_tensor("sgout", [128, NM * TPC], F32, kind="ExternalOutput").ap()

    x_view = x_d.rearrange("(a p) c -> p a c", p=128)      # [128, 16, 1024]
    out_view = out_d.rearrange("(a p) c -> p a c", p=128)
    w2t_view = w2t_d.rearrange("(a p) c -> p a c", p=128)  # [128, 16, 1024]

    with tile.TileContext(nc) as tc:
        with (
            tc.tile_pool(name="consts", bufs=1) as cpool,
            tc.tile_pool(name="weights", bufs=1) as wpool,
            tc.tile_pool(name="w2s", bufs=3) as w2pool,
            tc.tile_pool(name="persist", bufs=1) as ppool,
            tc.tile_pool(name="xin", bufs=3) as xpool,
            tc.tile_pool(name="sq", bufs=1) as sqpool,
            tc.tile_pool(name="xt", bufs=2) as xtpool2,
            tc.tile_pool(name="xtT", bufs=2) as xtpool,
            tc.tile_pool(name="xb", bufs=4) as xbpool,
            tc.tile_pool(name="h2", bufs=2) as h2pool,
            tc.tile_pool(name="kk2", bufs=2) as kkpool,
            tc.tile_pool(name="gat", bufs=2) as gpool,
            tc.tile_pool(name="hrelu", bufs=2) as hrpool,
            tc.tile_pool(name="ostage", bufs=3) as opool,
            tc.tile_pool(name="pp_small", bufs=1, space="PSUM") as pps,    # raw9 + aux (1 bank)
            tc.tile_pool(name="pp_h", bufs=2, space="PSUM") as pph,        # ph/xpose/pk (2 banks)
            tc.tile_pool(name="pp_os", bufs=4, space="PSUM") as ppos,      # po/psh (4 banks)
        ):
            # ---- constants / resident weights ----
            ident = cpool.tile([128, 128], F32, tag="ident")
            nc.sync.dma_start(ident[:], ident_d)
            sel = cpool.tile([128, E], F32, tag="sel")
            nc.sync.dma_start(sel[:], sel_d)
            kb = cpool.tile([128, JS], F32, tag="kb")
            nc.sync.dma_start(kb[:], kb_d)
            gwt = cpool.tile([128, CC, 9], F32, tag="gwt")
            nc.sync.dma_start(gwt[:], gwt_d.rearrange("(a p) e -> p a e", p=128))
            ones_col = cpool.tile([128, 1], F32, tag="ones")
            nc.vector.memset(ones_col[:], 1.0)

            w1t = wpool.tile([128, CC, H], F32R, tag="w1t")
            nc.sync.dma_start(w1t[:], w1t_d.rearrange("(a p) h -> p a h", p=128))
            kwt = wpool.tile([128, CC, HS], F32R, tag="kwt")
            nc.sync.dma_start(kwt[:], kwt_d.rearrange("(a p) h -> p a h", p=128))
            vwt = wpool.tile([128, JS, C], F32R, tag="vwt")
            nc.sync.dma_start(vwt[:], vwt_d.rearrange("(a p) c -> p a c", p=128))

            ge_all = ppool.tile([128, NM * TPC], F32, tag="ge_all")
            sg_all = ppool.tile([128, NM * TPC], F32, tag="sg_all")
            aux_oh = ppool.tile([128, E], F32, tag="aux_oh")
            aux_sc = ppool.tile([128, E], F32, tag="aux_sc")
            nc.vector.memset(aux_oh[:], 0.0)
            nc.vector.memset(aux_sc[:], 0.0)

            def prep_chunk(nm):
                """Load + rmsnorm one 256-token chunk; transpose to [c,n]; exact-fp32 gate matmul."""
                xtT = xtpool.tile([128, TPC, CC, 128], F32R, tag="xtT")
                raw_s = gpool.tile([128, TPC, 9], F32, tag="raw_s")
                ssq = gpool.tile([128, TPC], F32, tag="ssq")
                c1 = gpool.tile([128, TPC], F32, tag="c1")
                xts = []
                for t in range(TPC):
                    x_t = xpool.tile([128, C], F32, tag="x")
                    nc.sync.dma_start(x_t[:], x_view[:, nm * TPC + t, :])
                    sq = sqpool.tile([128, C], F32, tag="sq")
                    nc.vector.tensor_mul(sq[:], x_t[:], x_t[:])
                    nc.vector.reduce_sum(ssq[:, t:t + 1], sq[:], axis=AXX)
                    xts.append(x_t)
                m_t = gpool.tile([128, TPC], F32, tag="m_t")
                nc.vector.tensor_scalar(m_t[:], ssq[:], 1.0 / C, EPS, ALU.mult, ALU.add)
                rm = gpool.tile([128, TPC], F32, tag="rm")
                nc.vector.reciprocal(rm[:], m_t[:])
                nc.scalar.sqrt(c1[:], rm[:])

                for t in range(TPC):
                    xt_t = xtpool2.tile([128, C], F32, tag="xt")
                    nc.vector.tensor_scalar(xt_t[:], xts[t][:], c1[:, t:t + 1], None, ALU.mult)
                    raw9p = pps.tile([128, 9], F32, tag="raw9")
                    for half in range(2):  # 4 cc-transposes per psum tile
                        ppx = pph.tile([128, 512], F32, tag="ph")
                        for k in range(4):
                            cc = half * 4 + k
                            nc.tensor.transpose(
                                ppx[:, k * 128:(k + 1) * 128],
                                xt_t[:, cc * 128:(cc + 1) * 128],
                                ident[:],
                            )
                        for k in range(4):
                            cc = half * 4 + k
                            xb = xbpool.tile([128, 128], F32, tag="xb")
                            nc.vector.tensor_copy(xb[:], ppx[:, k * 128:(k + 1) * 128])
                            nc.scalar.copy(xtT[:, t, cc, :], ppx[:, k * 128:(k + 1) * 128])
                            nc.tensor.matmul(
                                raw9p[:], xb[:], gwt[:, cc, :],
                                start=(cc == 0), stop=(cc == CC - 1),
                            )
                    nc.scalar.copy(raw_s[:, t, :], raw9p[:])
                return xtT, raw_s

            def gating(nm, raw_s):
                """Top-2 softmax gating + aux accumulation on [128, TPC, 8]."""
                raw8 = raw_s[:, :, 0:8]
                m1 = gpool.tile([128, TPC], F32, tag="m1")
                nc.vector.reduce_max(m1[:], raw8, axis=AXX)
                m1b = m1[:].unsqueeze(2).broadcast_to([128, TPC, 8])
                eq1 = gpool.tile([128, TPC, 8], F32, tag="eq1")
                nc.vector.tensor_tensor(eq1[:], raw8, m1b, ALU.is_equal)
                masked = gpool.tile([128, TPC, 8], F32, tag="masked")
                nc.vector.scalar_tensor_tensor(masked[:], eq1[:], -1e9, raw8, ALU.mult, ALU.add)
                m2 = gpool.tile([128, TPC], F32, tag="m2")
                nc.vector.reduce_max(m2[:], masked[:], axis=AXX)

                sub = gpool.tile([128, TPC, 8], F32, tag="sub")
                nc.vector.tensor_tensor(sub[:], raw8, m1b, ALU.subtract)
                expsh = gpool.tile([128, TPC, 8], F32, tag="expsh")
                nc.scalar.activation(expsh[:], sub[:], ACTF.Exp)
                z_t = gpool.tile([128, TPC], F32, tag="z_t")
                nc.vector.reduce_sum(z_t[:], expsh[:], axis=AXX)
                rz = gpool.tile([128, TPC], F32, tag="rz")
                nc.vector.reciprocal(rz[:], z_t[:])

                d21 = gpool.tile([128, TPC], F32, tag="d21")
                nc.vector.tensor_tensor(d21[:], m2[:], m1[:], ALU.subtract)
                e2 = gpool.tile([128, TPC], F32, tag="e2")
                nc.scalar.activation(e2[:], d21[:], ACTF.Exp)
                s2 = gpool.tile([128, TPC], F32, tag="s2")
                nc.vector.tensor_tensor(s2[:], e2[:], rz[:], ALU.mult)
                den = gpool.tile([128, TPC], F32, tag="den")
                nc.vector.tensor_tensor(den[:], rz[:], s2[:], ALU.add)
                nc.vector.tensor_scalar(den[:], den[:], 1e-6, None, ALU.add)
                rden = gpool.tile([128, TPC], F32, tag="rden")
                nc.vector.reciprocal(rden[:], den[:])
                tw1 = gpool.tile([128, TPC], F32, tag="tw1")
                nc.vector.tensor_tensor(tw1[:], rz[:], rden[:], ALU.mult)
                tw2 = gpool.tile([128, TPC], F32, tag="tw2")
                nc.vector.tensor_tensor(tw2[:], s2[:], rden[:], ALU.mult)

                eq2 = gpool.tile([128, TPC, 8], F32, tag="eq2")
                m2b = m2[:].unsqueeze(2).broadcast_to([128, TPC, 8])
                nc.vector.tensor_tensor(eq2[:], raw8, m2b, ALU.is_equal)

                g1 = gpool.tile([128, TPC, 8], F32, tag="g1")
                nc.vector.tensor_tensor(
                    g1[:], eq1[:], tw1[:].unsqueeze(2).broadcast_to([128, TPC, 8]), ALU.mult)
                g2 = gpool.tile([128, TPC, 8], F32, tag="g2")
                nc.vector.tensor_tensor(
                    g2[:], eq2[:], tw2[:].unsqueeze(2).broadcast_to([128, TPC, 8]), ALU.mult)
                gmat = gpool.tile([128, TPC, 8], F32, tag="gmat")
                nc.vector.tensor_tensor(gmat[:], g1[:], g2[:], ALU.add)

                gsel = gpool.tile([128, TPC, 8], F32, tag="gsel")
                selb = sel[:].unsqueeze(1).broadcast_to([128, TPC, 8])
                nc.vector.tensor_tensor(gsel[:], gmat[:], selb, ALU.mult)
                nc.vector.reduce_sum(ge_all[:, nm * TPC:(nm + 1) * TPC], gsel[:], axis=AXX)

                nc.scalar.activation(sg_all[:, nm * TPC:(nm + 1) * TPC], raw_s[:, :, 8], ACTF.Sigmoid)

                oh = gpool.tile([128, TPC, 8], F32, tag="oh")
                nc.vector.tensor_tensor(oh[:], eq1[:], eq2[:], ALU.add)
                scor = gpool.tile([128, TPC, 8], F32, tag="scor")
                nc.vector.tensor_tensor(
                    scor[:], expsh[:], rz[:].unsqueeze(2).broadcast_to([128, TPC, 8]), ALU.mult)
                for t in range(TPC):
                    nc.vector.scalar_tensor_tensor(
                        aux_oh[:], oh[:, t, :], 0.5, aux_oh[:], ALU.mult, ALU.add)
                    nc.vector.tensor_tensor(aux_sc[:], aux_sc[:], scor[:, t, :], ALU.add)

            def up_proj(xtT):
                """Expert up-projection + relu^2 into f32r h2 [128, JJ, TPC*128]."""
                h2 = h2pool.tile([128, JJ, TPC * 128], F32R, tag="h2")
                for jh in range(JJ // 2):
                    ph = pph.tile([128, 2, TPC * 128], F32, tag="ph")
                    for k in range(2):
                        jj = jh * 2 + k
                        for cc in range(CC):
                            nc.tensor.matmul(
                                ph[:, k, :].rearrange("p (t n) -> p t n", t=TPC),
                                w1t[:, cc, jj * 128:(jj + 1) * 128],
                                xtT[:, :, cc, :],
                                start=(cc == 0), stop=(cc == CC - 1),
                            )
                    hr = hrpool.tile([128, 2, TPC * 128], F32, tag="hrelu")
                    nc.scalar.activation(hr[:], ph[:], ACTF.Relu)
                    nc.vector.tensor_tensor(h2[:, jh * 2:(jh + 1) * 2, :], hr[:], hr[:], ALU.mult)
                return h2

            def shared_up(xtT):
                """Shared-expert H-slice up-projection: relu(x@kwT + kb)^2 -> kk2."""
                pk = pph.tile([128, 2, TPC * 128], F32, tag="ph")
                for js in range(JS):
                    for cc in range(CC):
                        nc.tensor.matmul(
                            pk[:, js, :].rearrange("p (t n) -> p t n", t=TPC),
                            kwt[:, cc, js * 128:(js + 1) * 128],
                            xtT[:, :, cc, :],
                            start=(cc == 0), stop=(cc == CC - 1),
                        )
                kr = hrpool.tile([128, 2, TPC * 128], F32, tag="hrelu")
                for js in range(JS):
                    nc.scalar.activation(kr[:, js, :], pk[:, js, :], ACTF.Relu, bias=kb[:, js:js + 1])
                kk2 = kkpool.tile([128, JS, TPC * 128], F32R, tag="kk2")
                nc.vector.tensor_tensor(kk2[:], kr[:], kr[:], ALU.mult)
                return kk2

            def down_combine(pair, h2s, kk2s):
                """w2t-streamed down-proj for a chunk pair + shared down + gated combine + store."""
                for ii in range(C // 512):
                    pos = [[None] * TPC for _ in range(2)]
                    for nm2 in range(2):
                        for s in range(TPC):
                            pos[nm2][s] = ppos.tile([128, 512], F32, tag="po", name=f"po_{nm2}_{s}")
                    for jj in range(JJ):
                        w2s = w2pool.tile([128, 512], F32R, tag="w2s")
                        nc.sync.dma_start(w2s[:], w2t_view[:, jj, ii * 512:(ii + 1) * 512])
                        for nm2 in range(2):
                            for s in range(TPC):
                                nc.tensor.matmul(
                                    pos[nm2][s][:],
                                    h2s[nm2][:, jj, s * 128:(s + 1) * 128],
                                    w2s[:],
                                    start=(jj == 0), stop=(jj == JJ - 1),
                                )
                    for nm2 in range(2):
                        for s in range(TPC):
                            tg = (pair * 2 + nm2) * TPC + s
                            t1 = opool.tile([128, 512], F32, tag="t1")
                            nc.vector.tensor_scalar(
                                t1[:], pos[nm2][s][:], ge_all[:, tg:tg + 1], None, ALU.mult)
                            psh = ppos.tile([128, 512], F32, tag="po")
                            for js in range(JS):
                                nc.tensor.matmul(
                                    psh[:],
                                    kk2s[nm2][:, js, s * 128:(s + 1) * 128],
                                    vwt[:, js, ii * 512:(ii + 1) * 512],
                                    start=(js == 0), stop=(js == JS - 1),
                                )
                            outt = opool.tile([128, 512], F32, tag="outt")
                            nc.vector.scalar_tensor_tensor(
                                outt[:], psh[:], sg_all[:, tg:tg + 1], t1[:], ALU.mult, ALU.add)
                            nc.sync.dma_start(out_view[:, tg, ii * 512:(ii + 1) * 512], outt[:])

            for pair in range(NPAIR):
                h2s = []
                kk2s = []
                for nm2 in range(2):
                    nm = pair * 2 + nm2
                    xtT, raw_s = prep_chunk(nm)
                    gating(nm, raw_s)
                    h2s.append(up_proj(xtT))
                    kk2s.append(shared_up(xtT))
                down_combine(pair, h2s, kk2s)

            # ======== aux loss epilogue ========
            poh = pps.tile([1, E], F32, tag="raw9")
            nc.tensor.matmul(poh[:], ones_col[:], aux_oh[:], start=True, stop=True)
            ohs = gpool.tile([1, E], F32, tag="ohs")
            nc.scalar.copy(ohs[:], poh[:])
            psc = pps.tile([1, E], F32, tag="raw9")
            nc.tensor.matmul(psc[:], ones_col[:], aux_sc[:], start=True, stop=True)
            prod = gpool.tile([1, E], F32, tag="prod")
            nc.vector.tensor_tensor(prod[:], ohs[:], psc[:], ALU.mult)
            auxv = gpool.tile([1, 1], F32, tag="auxv")
            nc.vector.reduce_sum(auxv[:], prod[:], axis=AXX)
            nc.vector.tensor_scalar(auxv[:], auxv[:], AUX_SCALE, None, ALU.mult)
            nc.sync.dma_start(aux_d, auxv[:])
            nc.sync.dma_start(sg_d, sg_all[:])

    nc.compile()
    return nc


_CACHE = {}


def _get_program():
    if "nc" not in _CACHE:
        _CACHE["nc"] = build_program()
    return _CACHE["nc"]


def make_in_maps(x, gate_w, w1, w2, shared_gate_w, k_w, k_b, v_w, v_b):
    x2 = np.ascontiguousarray(np.asarray(x).reshape(N, C), dtype=np.float32)
    gw9 = np.ascontiguousarray(
        np.concatenate([np.asarray(gate_w), np.asarray(shared_gate_w)], axis=0).T, dtype=np.float32
    )  # [C, 9]
    ident = np.eye(128, dtype=np.float32)
    w1 = np.asarray(w1)
    w2 = np.asarray(w2)
    k_w = np.asarray(k_w)
    k_b = np.asarray(k_b)
    v_w = np.asarray(v_w)
    in_maps = []
    for e in range(NCORES):
        sl = slice(e * HS, (e + 1) * HS)
        sel = np.zeros((128, E), np.float32)
        sel[:, e] = 1.0
        kb = np.ascontiguousarray(k_b[sl].reshape(JS, 128).T, dtype=np.float32)  # [128, JS]
        in_maps.append({
            "x": x2,
            "w1t": np.ascontiguousarray(w1[e].T, dtype=np.float32),        # [C, H]
            "w2t": np.ascontiguousarray(w2[e].T, dtype=np.float32),        # [H, C]
            "kwt": np.ascontiguousarray(k_w[sl].T, dtype=np.float32),      # [C, HS]
            "vwt": np.ascontiguousarray(v_w[:, sl].T, dtype=np.float32),   # [HS, C]
            "kb": kb,
            "gwt": gw9,
            "sel": sel,
            "ident": ident,
        })
    return in_maps


def kernel(x, gate_w, w1, w2, shared_gate_w, k_w, k_b, v_w, v_b):
    nc = _get_program()
    in_maps = make_in_maps(x, gate_w, w1, w2, shared_gate_w, k_w, k_b, v_w, v_b)
    r = run_bass_kernel_spmd(nc, in_maps, list(range(NCORES)))
    out = np.zeros((N, C), np.float64)
    for e in range(NCORES):
        out += r.results[e]["out"].astype(np.float64)
    v_b = np.asarray(v_b)
    if np.any(v_b):
        # device ignores v_b; add sigmoid-gated bias on host (v_b is all-zero in practice)
        sg = r.results[0]["sgout"]  # [128, 16]; token a*128+p at [p, a]
        sg_tok = sg.T.reshape(N)
        out += sg_tok[:, None].astype(np.float64) * v_b[None, :].astype(np.float64)
    aux = np.float32(r.results[0]["aux"][0, 0])
    return out.astype(np.float32).reshape(1, N, C), aux
